# revision 1
# baseline (speedup 1.0000x reference)
"""TRN2 Bass kernel for nn_Attention_65283502899297 (sparse_attention).

Per batch element b (one per NeuronCore, 8 cores):
    q = Wq x, k = Wk x, v = Wv x           (1x1 conv, x: [384, 16384])
    qh, kh l2-normalized over hw; per head h (8 heads x 48 ch):
    A_h = softmax(qn_h kn_h^T / sqrt(hw)); out_h = A_h v_h

Algebraic restructure (the whole point of this kernel):
    G = x x^T                  [384, 384]   (one big matmul over hw)
    E_q = G Wq^T, E_k = G Wk^T; S^T_h = (Wk' E_q')_hh  (48x48 blocks)
    nq^2 = diag(Wq G Wq^T) = colsum(Wq^T o E_q)  (ones-matmul)
    logits^T = diag(rk) S^T diag(rq/sqrt(hw));  A^T = softmax over partitions
    M^T = Wv^T blockdiag(A)^T  (placement matmuls build blockdiag(A)^T)
    out = M x                  (second big matmul)
So v/q/k are never materialized and x is read from HBM exactly once.
G is computed upper-triangular only (symmetric) and completed by
transpose-matmuls. Raw Bass with explicit semaphores; every engine
instruction carries <=1 wait condition (walrus limit on this toolchain).
"""
import sys
sys.path.insert(0, '/opt/trn_rl_repo')

import numpy as np
import concourse.bass as bass
from concourse import mybir
from concourse.bass_utils import run_bass_kernel_spmd

f32 = mybir.dt.float32
bf16 = mybir.dt.bfloat16
AF = mybir.ActivationFunctionType

C = 384            # channels
NH, HC = 8, 48     # heads, head channels
CC = 3             # 128-row chunks of C
WIN = 2048         # hw window (columns) per resident x tile
NB_XT = 4          # xT sbuf buffers
NB_PXT = 3         # xT psum buffers
NOB = 4            # out staging buffers


def build_bass(nwin=8, stop_after='full'):
    hw = WIN * nwin
    nchunk = hw // 128
    cpw = WIN // 128                 # chunks per window (16)
    scale_sq = float(hw)             # rq = rsqrt(hw * nq^2) = 1/(nq*sqrt(hw))

    nc = bass.Bass()
    x_d = nc.dram_tensor("x", [C, hw], f32, kind="ExternalInput")
    w_d = nc.dram_tensor("w", [3 * C, C], f32, kind="ExternalInput")
    out_d = nc.dram_tensor("out", [C, hw], f32, kind="ExternalOutput")

    # placement matrices: P_{h,j}[s, p] = 1 iff p == 48h + s - 128j
    pm_list = []   # (j, h, base)
    for h in range(NH):
        lo, hi = 48 * h, 48 * h + 48
        for j in range(CC):
            if lo < 128 * (j + 1) and hi > 128 * j:
                pm_list.append((j, h, 48 * h - 128 * j))
    n_pm = len(pm_list)
    ngrp = CC * nwin                 # phase-4 (m, w) groups

    # ---- symbolic tick tables (single source of truth) ----
    A = {}
    for i, name in enumerate(
        ["g0", "g1p", "g2p", "sym1", "sym2", "sym3",
         "eq0", "eq1", "eq2", "ek0", "ek1", "ek2",
         "srq", "srk", "exp", "ab0", "ab1", "ab2"]):
        A[name] = i + 1
    P = {}
    for i, name in enumerate(
        ["symt1", "symt2", "symt3",
         "meq0", "meq1", "meq2", "mek0", "mek1", "mek2",
         "nq0", "nq1", "nq2", "nk0", "nk1", "nk2",
         "bcq", "bck", "st", "den", "bcr", "place", "mt0", "mt1", "mt2"]):
        P[name] = i + 1
    D = {}
    for i, name in enumerate(
        ["consts", "zq0", "zq1", "zq2", "zk0", "zk1", "zk2", "rq", "rk",
         "eqp0", "eqp1", "eqp2", "wkp0", "wkp1", "wkp2", "r", "rb",
         "nt0", "nt1", "nt2"]):
        D[name] = i + 1

    eg_bank = [5, 6, 4, 5, 6, 4]     # E-group psum banks
    eg_name = ["eq0", "eq1", "eq2", "ek0", "ek1", "ek2"]
    mt_bank = [5, 6, 5]              # M^T group psum banks

    from contextlib import ExitStack
    ctx = ExitStack()
    with ctx:
        _n = [0]

        def sbt(shape, dt):
            _n[0] += 1
            return ctx.enter_context(nc.sbuf_tensor(f"t{_n[0]}", shape, dt))

        def ps():
            _n[0] += 1
            return ctx.enter_context(
                nc.psum_tensor(f"p{_n[0]}", [128, 512], f32))

        sem = lambda name: ctx.enter_context(nc.semaphore(name))

        xw = [[sbt([128, WIN], bf16) for _ in range(nwin)] for _ in range(CC)]
        w_bf = [sbt([128, C], bf16) for _ in range(9)]
        wv = [sbt([128, C], f32) for _ in range(CC)]
        wT = [sbt([128, 3 * C], bf16) for _ in range(CC)]
        ident = sbt([128, 128], bf16)
        ones_col = sbt([128, 1], f32)
        ones_row = sbt([1, 128], f32)
        zrow = sbt([1, 128], f32)
        xT = [sbt([128, C], bf16) for _ in range(NB_XT)]
        g_sb = [sbt([128, C], bf16) for _ in range(CC)]
        eq_sb = [sbt([128, C], f32) for _ in range(CC)]
        ek_sb = [sbt([128, C], f32) for _ in range(CC)]
        zq_sb = [sbt([128, C], f32) for _ in range(CC)]
        zk_sb = [sbt([128, C], f32) for _ in range(CC)]
        srq = sbt([1, C], f32)
        srk = sbt([1, C], f32)
        rq = sbt([1, C], f32)
        rk = sbt([1, C], f32)
        eqp = [sbt([128, C], bf16) for _ in range(CC)]
        wkp = [sbt([128, C], bf16) for _ in range(CC)]
        expLT = sbt([48, C], f32)
        r_row = sbt([1, C], f32)
        rb_sb = sbt([128, C], f32)
        pmat = [sbt([48, 128], f32) for _ in pm_list]
        ablkT = [sbt([128, C], f32) for _ in range(CC)]
        ntb = [sbt([128, C], bf16) for _ in range(CC)]
        outb = [sbt([128, WIN], f32) for _ in range(NOB)]

        pb = [ps() for _ in range(8)]
        # pb0-2: G accum (ph1), AblkT (ph3), pout even groups (ph4)
        # pb3-5: pxT (ph1); pb3: sym-T + bcast q + bcast r; pb4: bcast k
        # pb5/6: E and M^T groups (alternating); pb6/7: wT (ph0)
        # pb7: nq/nk + ST; pb6: softmax denominator; ph4: 0-3 / 4-7 alternate

        s_xw = [sem(f"s_x{w}") for w in range(nwin)]
        s_w = sem("s_w")
        s_pl = sem("s_pl")
        s_wt = sem("s_wt")
        s_wte = sem("s_wte")
        s_tmm = sem("s_tmm")
        s_te = sem("s_te")
        s_g = sem("s_g")
        s_a2 = sem("s_a2")
        s_p2 = sem("s_p2")
        s_d2 = sem("s_d2")
        s_mm4 = sem("s_mm4")
        s_o4 = sem("s_o4")
        s_stb = [sem(f"s_st{i}") for i in range(NOB)]

        with nc.Block() as block:
            # ------------- gpsimd: loads + constants + odd stores -------
            @block.gpsimd
            def _(g):
                for j in range(9):
                    g.dma_start(out=w_bf[j][:, :],
                                in_=w_d[128 * j:128 * (j + 1), :]
                                ).then_inc(s_w, 16)
                for k in range(CC):
                    g.dma_start(out=wv[k][:, :],
                                in_=w_d[768 + 128 * k:768 + 128 * (k + 1), :]
                                ).then_inc(s_w, 16)
                g.memset(ident[:, :], 0.0).then_inc(s_pl, 1)
                for i in range(n_pm):
                    g.memset(pmat[i][:, :], 0.0).then_inc(s_pl, 1)
                g.wait_ge(s_pl, 1 + n_pm)
                g.affine_select(out=ident[:, :], in_=ident[:, :],
                                compare_op=mybir.AluOpType.not_equal,
                                fill=1.0, base=0, pattern=[[-1, 128]],
                                channel_multiplier=1).then_inc(s_pl, 1)
                for i, (j, h, base) in enumerate(pm_list):
                    g.affine_select(out=pmat[i][:, :], in_=pmat[i][:, :],
                                    compare_op=mybir.AluOpType.not_equal,
                                    fill=1.0, base=base, pattern=[[-1, 128]],
                                    channel_multiplier=1).then_inc(s_pl, 1)
                # x loads (cast fp32->bf16), window-major, paced 3 deep
                for w in range(nwin):
                    if w >= 3:
                        g.wait_ge(s_xw[w - 3], 48)
                    for k in range(CC):
                        g.dma_start(out=xw[k][w][:, :],
                                    in_=x_d[128 * k:128 * (k + 1),
                                            WIN * w:WIN * (w + 1)]
                                    ).then_inc(s_xw[w], 16)
                if stop_after != 'full':
                    return
                # odd phase-4 groups stored via SWDGE
                for grp in range(1, ngrp, 2):
                    m, w = grp // nwin, grp % nwin
                    g.wait_ge(s_o4, 4 * (grp + 1))
                    g.dma_start(
                        out=out_d[128 * m:128 * (m + 1),
                                  WIN * w:WIN * (w + 1)],
                        in_=outb[grp % NOB][:, :]).then_inc(
                            s_stb[grp % NOB], 16)
                for i in range(1, NOB, 2):
                    cnt = len([g for g in range(1, ngrp, 2) if g % NOB == i])
                    g.wait_ge(s_stb[i], 16 * cnt)

            # ------------- DVE: consts + phase2/3 elementwise -----------
            @block.vector
            def _(d):
                dv = [0]

                def dinc(inst, name):
                    dv[0] += 1
                    assert D[name] == dv[0], (name, dv[0])
                    inst.then_inc(s_d2, 1)

                d.memset(ones_col[:, :], 1.0)
                d.memset(ones_row[:, :], 1.0)
                dinc(d.memset(zrow[:, :], 0.0), "consts")
                if stop_after == 'ph1':
                    return
                for k in range(CC):
                    d.wait_ge(s_a2, A[f"eq{k}"])
                    dinc(d.tensor_mul(zq_sb[k][:, :], eq_sb[k][:, :],
                                      wT[k][:, 0:C]), f"zq{k}")
                for k in range(CC):
                    d.wait_ge(s_a2, A[f"ek{k}"])
                    dinc(d.tensor_mul(zk_sb[k][:, :], ek_sb[k][:, :],
                                      wT[k][:, C:2 * C]), f"zk{k}")
                d.wait_ge(s_a2, A["srq"])
                dinc(d.reciprocal(rq[:, :], srq[:, :]), "rq")
                d.wait_ge(s_a2, A["srk"])
                dinc(d.reciprocal(rk[:, :], srk[:, :]), "rk")
                d.wait_ge(s_p2, P["bck"])
                for k in range(CC):
                    dinc(d.tensor_mul(eqp[k][:, :], eq_sb[k][:, :],
                                      pb[3][:, 0:C]), f"eqp{k}")
                for k in range(CC):
                    dinc(d.tensor_mul(wkp[k][:, :], wT[k][:, C:2 * C],
                                      pb[4][:, 0:C]), f"wkp{k}")
                d.wait_ge(s_p2, P["den"])
                dinc(d.reciprocal(r_row[:, :], pb[6][0:1, 0:C]), "r")
                d.wait_ge(s_p2, P["bcr"])
                dinc(d.tensor_copy(rb_sb[:, :], pb[3][:, 0:C]), "rb")
                d.wait_ge(s_d2, D["rb"])
                # Nt = M^T_unnorm column-scaled by 1/den (cast bf16)
                for m in range(CC):
                    d.wait_ge(s_p2, P[f"mt{m}"])
                    dinc(d.tensor_mul(ntb[m][:, :],
                                      pb[mt_bank[m]][:, 0:C], rb_sb[:, :]),
                         f"nt{m}")

            # ------------- PE: every matmul -----------------------------
            @block.tensor
            def _(t):
                pe2 = [0]

                def pinc(inst, name):
                    pe2[0] += 1
                    assert P[name] == pe2[0], (name, pe2[0])
                    inst.then_inc(s_p2, 1)

                # phase 0: wT via matmul-transpose (psum pb6/pb7)
                t.wait_ge(s_pl, 2 + n_pm)
                t.wait_ge(s_w, 192)
                for jk in range(27):
                    j, k = jk // 3, jk % 3
                    if jk >= 2:
                        t.wait_ge(s_wte, jk - 1)
                    t.matmul(pb[6 + jk % 2][:, 0:128],
                             w_bf[j][:, 128 * k:128 * (k + 1)], ident[:, :],
                             start=True, stop=True).then_inc(s_wt, 1)

                # phase 1: x transposes + triangular Gram, pipelined depth 2
                def xpose(i):
                    w, c = i // cpw, i % cpw
                    if c == 0:
                        t.wait_ge(s_xw[w], 48)
                    for k in range(CC):
                        mm = t.matmul(
                            pb[3 + i % NB_PXT][:, 128 * k:128 * (k + 1)],
                            xw[k][w][:, 128 * c:128 * (c + 1)],
                            ident[:, :], start=True, stop=True)
                        if k == CC - 1:
                            mm.then_inc(s_tmm, 1)

                def gram(i):
                    t.wait_ge(s_te, i + 1)
                    for m in range(CC):
                        mm = t.matmul(pb[m][:, 0:C - 128 * m],
                                      xT[i % NB_XT][:, 128 * m:128 * (m + 1)],
                                      xT[i % NB_XT][:, 128 * m:C],
                                      start=(i == 0), stop=(i == nchunk - 1))
                        if m == CC - 1:
                            mm.then_inc(s_g, 1)

                for i in range(nchunk + 2):
                    if i < nchunk:
                        xpose(i)
                    if i >= 2:
                        gram(i - 2)
                if stop_after == 'ph1':
                    return

                # symmetry completion: 3 transpose-MMs into pb3
                t.wait_ge(s_te, nchunk)
                t.wait_ge(s_a2, A["g0"])
                pinc(t.matmul(pb[3][:, 0:128], g_sb[0][:, 128:256],
                              ident[:, :], start=True, stop=True), "symt1")
                pinc(t.matmul(pb[4][:, 0:128], g_sb[0][:, 256:384],
                              ident[:, :], start=True, stop=True), "symt2")
                t.wait_ge(s_a2, A["g1p"])
                pinc(t.matmul(pb[5][:, 0:128], g_sb[1][:, 256:384],
                              ident[:, :], start=True, stop=True), "symt3")

                # phase 2: E_q / E_k (bf16), banks alternate pb5/pb6
                t.wait_ge(s_wte, 27)
                for grp in range(6):
                    src_off = 0 if grp < CC else C
                    m = grp % CC
                    if grp == 0:
                        t.wait_ge(s_a2, A["sym3"])
                    if grp >= 3:
                        t.wait_ge(s_a2, A[eg_name[grp - 3]])
                    for k in range(CC):
                        mm = t.matmul(pb[eg_bank[grp]][:, 0:C],
                                      g_sb[k][:, 128 * m:128 * (m + 1)],
                                      wT[k][:, src_off:src_off + C],
                                      start=(k == 0), stop=(k == CC - 1))
                    pinc(mm, f"m{eg_name[grp]}")
                # norms (fp32 ones-matmuls into pb7)
                for k in range(CC):
                    t.wait_ge(s_d2, D[f"zq{k}"])
                    pinc(t.matmul(pb[7][0:1, 0:C], ones_col[:, 0:1],
                                  zq_sb[k][:, :], start=(k == 0),
                                  stop=(k == CC - 1)), f"nq{k}")
                for k in range(CC):
                    t.wait_ge(s_d2, D[f"zk{k}"])
                    if k == 0:
                        t.wait_ge(s_a2, A["ek1"])   # pb6 free of E use
                    pinc(t.matmul(pb[6][0:1, 0:C], ones_col[:, 0:1],
                                  zk_sb[k][:, :], start=(k == 0),
                                  stop=(k == CC - 1)), f"nk{k}")
                # broadcasts of rq (pb3) and rk (pb4)
                t.wait_ge(s_d2, D["rq"])
                pinc(t.matmul(pb[3][:, 0:C], ones_row[0:1, :], rq[:, :],
                              start=True, stop=True), "bcq")
                t.wait_ge(s_d2, D["rk"])
                t.wait_ge(s_a2, A["ek2"])   # pb4 free of E-group eviction
                pinc(t.matmul(pb[4][:, 0:C], ones_row[0:1, :], rk[:, :],
                              start=True, stop=True), "bck")
                # S^T per head (bf16) into pb7
                t.wait_ge(s_d2, D["wkp2"])
                t.wait_ge(s_a2, A["srq"])
                for h in range(NH):
                    for k in range(CC):
                        mm = t.matmul(pb[7][0:48, 48 * h:48 * (h + 1)],
                                      wkp[k][:, 48 * h:48 * (h + 1)],
                                      eqp[k][:, 48 * h:48 * (h + 1)],
                                      start=(k == 0), stop=(k == CC - 1))
                pinc(mm, "st")
                # softmax denominator (fp32) into pb6
                t.wait_ge(s_a2, A["exp"])
                pinc(t.matmul(pb[6][0:1, 0:C], ones_col[0:48, 0:1],
                              expLT[:, :], start=True, stop=True), "den")
                # broadcast r over all partitions into pb3
                t.wait_ge(s_d2, D["r"])
                pinc(t.matmul(pb[3][:, 0:C], ones_row[0:1, :],
                              r_row[:, :], start=True, stop=True), "bcr")
                # phase 3: blockdiag(exp)^T via placement matmuls (fp32, pb0-2)
                t.wait_ge(s_pl, 2 + 2 * n_pm)
                last_of_j = {}
                for i, (j, h, base) in enumerate(pm_list):
                    last_of_j[j] = i
                for j in range(CC):
                    t.matmul(pb[j][:, 0:C], zrow[0:1, :], srq[:, :],
                             start=True, stop=False)
                for i, (j, h, base) in enumerate(pm_list):
                    mm = t.matmul(pb[j][:, 48 * h:48 * (h + 1)],
                                  pmat[i][0:48, :],
                                  expLT[0:48, 48 * h:48 * (h + 1)],
                                  start=False, stop=(last_of_j[j] == i))
                pinc(mm, "place")
                # M^T = Wv^T AblkT (fp32), banks pb5/pb6/pb5
                for m in range(CC):
                    if m == 0:
                        t.wait_ge(s_a2, A["ab2"])
                    if m == 2:
                        t.wait_ge(s_d2, D["nt0"])
                    for kv in range(CC):
                        mm = t.matmul(pb[mt_bank[m]][:, 0:C],
                                      wv[kv][:, 128 * m:128 * (m + 1)],
                                      ablkT[kv][:, :],
                                      start=(kv == 0), stop=(kv == CC - 1))
                    pinc(mm, f"mt{m}")
                if stop_after == 'ph3':
                    return

                # phase 4: out = M x
                t.wait_ge(s_a2, A["ab2"])
                t.wait_ge(s_d2, D["nt2"])
                for grp in range(ngrp):
                    m, w = grp // nwin, grp % nwin
                    b0 = 4 * (grp % 2)
                    if grp >= 2:
                        t.wait_ge(s_o4, 4 * (grp - 1))
                    for k in range(CC):
                        for ns in range(4):
                            mm = t.matmul(pb[b0 + ns][:, 0:512],
                                          ntb[k][:, 128 * m:128 * (m + 1)],
                                          xw[k][w][:, 512 * ns:512 * (ns + 1)],
                                          start=(k == 0), stop=(k == CC - 1))
                            if k == CC - 1:
                                mm.then_inc(s_mm4, 1)

            # ------------- ACT: evictions + exp + sqrt ------------------
            @block.scalar
            def _(s):
                a2 = [0]

                def ainc(inst, name):
                    a2[0] += 1
                    assert A[name] == a2[0], (name, a2[0])
                    inst.then_inc(s_a2, 1)

                for jk in range(27):
                    j, k = jk // 3, jk % 3
                    s.wait_ge(s_wt, jk + 1)
                    s.copy(wT[k][:, 128 * j:128 * (j + 1)],
                           pb[6 + jk % 2][:, 0:128]).then_inc(s_wte, 1)
                for i in range(nchunk):
                    s.wait_ge(s_tmm, i + 1)
                    if i >= NB_XT:
                        s.wait_ge(s_g, i - NB_XT + 1)
                    s.copy(xT[i % NB_XT][:, :],
                           pb[3 + i % NB_PXT][:, 0:C]).then_inc(s_te, 1)
                if stop_after == 'ph1':
                    return
                # G evictions (cast bf16): g0 full; g1 cols 128:; g2 cols 256:
                s.wait_ge(s_g, nchunk)
                ainc(s.copy(g_sb[0][:, :], pb[0][:, 0:C]), "g0")
                ainc(s.copy(g_sb[1][:, 128:C], pb[1][:, 0:C - 128]), "g1p")
                ainc(s.copy(g_sb[2][:, 256:C], pb[2][:, 0:C - 256]), "g2p")
                # symmetry-completion evictions from pb3
                s.wait_ge(s_p2, P["symt1"])
                ainc(s.copy(g_sb[1][:, 0:128], pb[3][:, 0:128]), "sym1")
                s.wait_ge(s_p2, P["symt2"])
                ainc(s.copy(g_sb[2][:, 0:128], pb[4][:, 0:128]), "sym2")
                s.wait_ge(s_p2, P["symt3"])
                ainc(s.copy(g_sb[2][:, 128:256], pb[5][:, 0:128]), "sym3")
                # E evictions
                for grp in range(6):
                    s.wait_ge(s_p2, P[f"m{eg_name[grp]}"])
                    dst = eq_sb[grp] if grp < CC else ek_sb[grp - CC]
                    ainc(s.copy(dst[:, :], pb[eg_bank[grp]][:, 0:C]),
                         eg_name[grp])
                # sqrt: srq = sqrt(hw*nq^2) = nq*sqrt(hw);  srk = nk
                s.wait_ge(s_p2, P["nq2"])
                ainc(s.activation(srq[:, :], pb[7][0:1, 0:C], AF.Sqrt,
                                  scale=scale_sq), "srq")
                s.wait_ge(s_p2, P["nk2"])
                ainc(s.activation(srk[:, :], pb[6][0:1, 0:C], AF.Sqrt,
                                  scale=1.0), "srk")
                # exp of logits^T
                s.wait_ge(s_p2, P["st"])
                ainc(s.activation(expLT[:, :], pb[7][0:48, 0:C], AF.Exp),
                     "exp")
                # ablkT evictions (fp32)
                s.wait_ge(s_p2, P["place"])
                for j in range(CC):
                    ainc(s.copy(ablkT[j][:, :], pb[j][:, 0:C]), f"ab{j}")
                if stop_after == 'ph3':
                    return
                # phase 4: out evictions
                for grp in range(ngrp):
                    b0 = 4 * (grp % 2)
                    if grp >= NOB:
                        s.wait_ge(s_stb[grp % NOB], 16 * (grp // NOB))
                    for ns in range(4):
                        s.wait_ge(s_mm4, 4 * grp + ns + 1)
                        s.copy(outb[grp % NOB][:, 512 * ns:512 * (ns + 1)],
                               pb[b0 + ns][:, 0:512]).then_inc(s_o4, 1)

            # ------------- SP: even phase-4 stores ----------------------
            @block.sync
            def _(sp):
                if stop_after != 'full':
                    return
                for grp in range(0, ngrp, 2):
                    m, w = grp // nwin, grp % nwin
                    sp.wait_ge(s_o4, 4 * (grp + 1))
                    sp.dma_start(
                        out=out_d[128 * m:128 * (m + 1),
                                  WIN * w:WIN * (w + 1)],
                        in_=outb[grp % NOB][:, :]).then_inc(
                            s_stb[grp % NOB], 16)
                for i in range(0, NOB, 2):
                    cnt = len([g for g in range(0, ngrp, 2) if g % NOB == i])
                    sp.wait_ge(s_stb[i], 16 * cnt)

    return nc


_cache = {}


def _get_nc(nwin=8):
    if nwin not in _cache:
        _cache[nwin] = build_bass(nwin)
    return _cache[nwin]


def kernel(x, w_qkv):
    """x: [8, 384, 128, 128] f32, w_qkv: [1152, 384] f32 ->
    out: [8, 384, 128, 128] f32. Batch-parallel over 8 NeuronCores."""
    x = np.ascontiguousarray(x, dtype=np.float32)
    w_qkv = np.ascontiguousarray(w_qkv, dtype=np.float32)
    B = x.shape[0]
    nc = _get_nc(8)
    in_maps = [{"x": x[b].reshape(C, WIN * 8), "w": w_qkv} for b in range(B)]
    res = run_bass_kernel_spmd(nc, in_maps, list(range(B)))
    out = np.stack([res.results[b]["out"] for b in range(B)])
    return out.reshape(x.shape).astype(np.float32)



# revision 10
# speedup vs baseline: 2.1081x; 2.1081x over previous
"""TRN2 Bass kernel for nn_Attention_65283502899297 (sparse_attention).

Deviation-form restructure. Per batch element b (one per NeuronCore):
    q = Wq x, k = Wk x, v = Wv x;  qh, kh l2-normalized over hw;
    A_h = softmax(qn_h kn_h^T / sqrt(hw));  out_h = A_h v_h.

Key numeric fact: logits = cos(q_c, k_d)/sqrt(hw) ~ N(0, 1/hw^1.5) are tiny
(|z| < 4e-4), so softmax(z) = (1 + dev)/den with dev = expm1(z) ~ z. Split:
    out = B[head(c)]/den_c  +  (M_dev x)[c]
where B = (sum of Wv rows per head) @ x is EXACT on the host (f32), and only
the deviation term M_dev x (~0.3% of out) runs on device. That term tolerates
fp8, so the two big matmuls use e4m3 DoubleRow (2 K-rows/pass, 0.5 cyc/row):
    G = x8 x8^T   from host-packed pair-transposed xt8 tiles (no PE transpose)
    out_dev = M_dev x8  with M_dev built from expm1(logits) via placement mms
Device outputs: out_dev (fp8, x32768) + den (f32). Host combines.
Host precomputes: x8 (e4m3), xt8 (pair-packed transpose), wT/wv (bf16), B.
Raw Bass, explicit semaphores, <=1 wait condition per engine instruction.
"""
import sys
sys.path.insert(0, '/opt/trn_rl_repo')

import numpy as np
import ml_dtypes
import concourse.bass as bass
from concourse import mybir
from concourse.bass_utils import run_bass_kernel_spmd

f32 = mybir.dt.float32
bf16 = mybir.dt.bfloat16
fp8 = mybir.dt.float8e4
AF = mybir.ActivationFunctionType
DR = mybir.MatmulPerfMode.DoubleRow
E4 = ml_dtypes.float8_e4m3
BF = ml_dtypes.bfloat16

C = 384            # channels
NH, HC = 8, 48     # heads, head channels
CC = 3             # 128-row chunks of C
HW = 16384
WIN = 2048         # hw window per x8 block
NWIN = 8
NC64 = 64          # 256-row gram chunks
SCALE = 32768.0
NOB = 4            # out staging buffers
NGRP = CC * NWIN   # phase-4 (m, w) groups


def build_bass(nwin=NWIN, stop_after='full'):
    # placement list: (j, h, base) with base = 48h - 128j, head h rows
    # intersecting c-chunk j
    pm_list = []
    for h in range(NH):
        lo, hi = HC * h, HC * (h + 1)
        for j in range(CC):
            if lo < 128 * (j + 1) and hi > 128 * j:
                pm_list.append((j, h, HC * h - 128 * j))

    # per-chunk written column ranges for ablkT (rest stays memset-0)
    ab_rng = []
    for j in range(CC):
        hs = [h for (jj, h, b) in pm_list if jj == j]
        ab_rng.append((HC * min(hs), HC * (max(hs) + 1)))

    eg_bank = [5, 6, 4, 5, 6, 4]
    eg_name = ["eq0", "eq1", "eq2", "ek0", "ek1", "ek2"]
    mt_bank = [5, 6, 5]

    # symbolic tick tables
    A = {}
    for i, name in enumerate(
        ["g0", "g1p", "g2p", "sym1", "sym2", "sym3",
         "eq0", "eq1", "eq2", "ek0", "ek1", "ek2",
         "srq", "srk", "exp", "denst", "ab0", "ab1", "ab2"]):
        A[name] = i + 1
    P = {}
    for i, name in enumerate(
        ["symt1", "symt2", "symt3",
         "meq0", "meq1", "meq2", "mek0", "mek1", "mek2",
         "nq0", "nq1", "nq2", "nk0", "nk1", "nk2",
         "bcq", "bck", "st", "den", "bcr", "place", "mt0", "mt1", "mt2"]):
        P[name] = i + 1
    D = {}
    for i, name in enumerate(
        ["consts", "zq0", "zq1", "zq2", "zk0", "zk1", "zk2", "rq", "rk",
         "eqp0", "eqp1", "eqp2", "wkp0", "wkp1", "wkp2",
         "expdev", "r", "rb", "nt0", "nt1", "nt2"]):
        D[name] = i + 1

    nc = bass.Bass()
    xt8_d = nc.dram_tensor("xt8", [128, NC64 * 2 * C], fp8, kind="ExternalInput")
    x8_d = nc.dram_tensor("x8", [C, HW], fp8, kind="ExternalInput")
    wt_d = nc.dram_tensor("wt", [C, 3 * C], bf16, kind="ExternalInput")
    wv_d = nc.dram_tensor("wv", [C, C], bf16, kind="ExternalInput")
    od_d = nc.dram_tensor("outdev", [C, HW], fp8, kind="ExternalOutput")
    den_d = nc.dram_tensor("den", [1, C], f32, kind="ExternalOutput")

    from contextlib import ExitStack
    ctx = ExitStack()
    with ctx:
        sbt = lambda name, shape, dt: ctx.enter_context(
            nc.sbuf_tensor(name, shape, dt))
        sem = lambda name: ctx.enter_context(nc.semaphore(name))

        xt8 = [sbt(f"xt8_{l}", [128, 16, C], fp8) for l in range(8)]
        # x8b blocks: window w chunk c at blk(w)+c with zero blocks at 12
        # (serves w<4) and 25 (serves w>=4) so pass-2 AP strides fit the
        # 16-bit ISA step field
        x8b = sbt("x8b", [128, 3 * NWIN + 2, WIN], fp8)
        blk = lambda w: 3 * w + (0 if w < 4 else 1)
        zblk = lambda w: 12 if w < 4 else 25
        wT = [sbt(f"wT{k}", [128, 3 * C], bf16) for k in range(CC)]
        wv = [sbt(f"wv{k}", [128, C], bf16) for k in range(CC)]
        identE = sbt("identE", [128, C], bf16)
        ones_cb = sbt("ones_cb", [128, 1], bf16)
        ones_cf = sbt("ones_cf", [128, 1], f32)
        ones_rb = sbt("ones_rb", [1, 128], bf16)
        g_sb = [sbt(f"g{k}", [128, C], bf16) for k in range(CC)]
        eq_sb = [sbt(f"eq{k}", [128, C], f32) for k in range(CC)]
        ek_sb = [sbt(f"ek{k}", [128, C], f32) for k in range(CC)]
        zq_sb = [sbt(f"zq{k}", [128, C], bf16) for k in range(CC)]
        zk_sb = [sbt(f"zk{k}", [128, C], bf16) for k in range(CC)]
        srq = sbt("srq", [1, C], f32)
        srk = sbt("srk", [1, C], f32)
        rq = sbt("rq", [1, C], bf16)
        rk = sbt("rk", [1, C], bf16)
        eqp = [sbt(f"eqp{k}", [128, C], bf16) for k in range(CC)]
        wkp = [sbt(f"wkp{k}", [128, C], bf16) for k in range(CC)]
        expLT = sbt("expLT", [48, C], f32)
        expdev = sbt("expdev", [48, C], bf16)
        den_sb = sbt("den_sb", [1, C], f32)
        r_row = sbt("r_row", [1, C], bf16)
        rb_sb = sbt("rb_sb", [128, C], f32)
        ablkT = [sbt(f"ab{j}", [128, C], bf16) for j in range(CC)]
        stA = sbt("stA", [128, 2, C], fp8)
        stB = sbt("stB", [128, 2, C], fp8)
        outb = [sbt(f"ob{i}", [128, WIN], fp8) for i in range(NOB)]

        pA = ctx.enter_context(nc.psum_tensor("pA", [128, 2048], f32))
        pB = ctx.enter_context(nc.psum_tensor("pB", [128, 2048], f32))

        def pb(i):
            t = pA if i < 4 else pB
            return t[:, 512 * (i % 4):512 * (i % 4) + 512]

        s_w = sem("s_w")
        s_pl = sem("s_pl")
        s_xt = [sem(f"s_xt{l}") for l in range(8)]
        s_x8 = [sem(f"s_x8w{w}") for w in range(NWIN)]
        s_g = sem("s_g")
        s_a2 = sem("s_a2")
        s_p2 = sem("s_p2")
        s_d2 = sem("s_d2")
        s_mm4 = sem("s_mm4")
        s_oa = sem("s_oa")     # ACT phase-4 evictions (even groups)
        s_od = sem("s_od")     # DVE phase-4 evictions (odd groups)
        s_stb = [sem(f"s_st{i}") for i in range(NOB)]
        s_den = sem("s_den")

        with nc.Block() as block:
            # ------------- gpsimd: big loads + identity -----------------
            @block.gpsimd
            def _(g):
                g.memset(identE[:, :], 0.0).then_inc(s_pl, 1)
                g.wait_ge(s_pl, 1)
                g.affine_select(out=identE[:, :], in_=identE[:, :],
                                compare_op=mybir.AluOpType.not_equal,
                                fill=1.0, base=128, pattern=[[-1, C]],
                                channel_multiplier=1).then_inc(s_pl, 1)
                for l in range(8):
                    g.dma_start(out=xt8[l][:, :, :],
                                in_=xt8_d[:, 6144 * l:6144 * (l + 1)]
                                ).then_inc(s_xt[l], 16)
                for w in range(NWIN):
                    g.dma_start(
                        out=x8b[:, blk(w):blk(w) + 3, :],
                        in_=x8_d[:, WIN * w:WIN * (w + 1)].rearrange(
                            "(c p) j -> p c j", c=CC)).then_inc(s_x8[w], 16)

            # ------------- PE: every matmul -----------------------------
            @block.tensor
            def _(t):
                pe2 = [0]

                def pinc(inst, name):
                    pe2[0] += 1
                    assert P[name] == pe2[0], (name, pe2[0])
                    inst.then_inc(s_p2, 1)

                # phase 1: triangular Gram, fp8 DoubleRow over 256-row chunks
                for c64 in range(NC64):
                    l, j = c64 // 8, c64 % 8
                    if j == 0:
                        t.wait_ge(s_xt[l], 16)
                    for m in range(CC):
                        mm = t.matmul(
                            pb(m)[:, 0:C - 128 * m],
                            xt8[l][:, 2 * j:2 * j + 2, 128 * m:128 * (m + 1)],
                            xt8[l][:, 2 * j:2 * j + 2, 128 * m:C],
                            start=(c64 == 0), stop=(c64 == NC64 - 1),
                            perf_mode=DR)
                    if c64 == NC64 - 1:
                        mm.then_inc(s_g, 1)
                if stop_after == 'ph1':
                    return

                # symmetry completion (3 transpose-by-identity matmuls)
                ident = identE[:, 128:256]
                t.wait_ge(s_pl, 2)
                t.wait_ge(s_a2, A["g0"])
                pinc(t.matmul(pb(3)[:, 0:128], g_sb[0][:, 128:256],
                              ident, start=True, stop=True), "symt1")
                pinc(t.matmul(pb(4)[:, 0:128], g_sb[0][:, 256:384],
                              ident, start=True, stop=True), "symt2")
                t.wait_ge(s_a2, A["g1p"])
                pinc(t.matmul(pb(5)[:, 0:128], g_sb[1][:, 256:384],
                              ident, start=True, stop=True), "symt3")

                # phase 2: E_q / E_k (bf16)
                t.wait_ge(s_w, 96)
                for grp in range(6):
                    src = 0 if grp < CC else C
                    m = grp % CC
                    if grp == 0:
                        t.wait_ge(s_a2, A["sym3"])
                    if grp >= 3:
                        t.wait_ge(s_a2, A[eg_name[grp - 3]])
                    for k in range(CC):
                        mm = t.matmul(pb(eg_bank[grp])[:, 0:C],
                                      g_sb[k][:, 128 * m:128 * (m + 1)],
                                      wT[k][:, src:src + C],
                                      start=(k == 0), stop=(k == CC - 1))
                    pinc(mm, f"m{eg_name[grp]}")
                # norms (bf16 ones-matmuls)
                for k in range(CC):
                    t.wait_ge(s_d2, D[f"zq{k}"])
                    pinc(t.matmul(pb(7)[0:1, 0:C], ones_cb[:, 0:1],
                                  zq_sb[k][:, :], start=(k == 0),
                                  stop=(k == CC - 1)), f"nq{k}")
                for k in range(CC):
                    t.wait_ge(s_d2, D[f"zk{k}"])
                    if k == 0:
                        t.wait_ge(s_a2, A["ek1"])
                    pinc(t.matmul(pb(6)[0:1, 0:C], ones_cb[:, 0:1],
                                  zk_sb[k][:, :], start=(k == 0),
                                  stop=(k == CC - 1)), f"nk{k}")
                # broadcasts of rq (pb3) / rk (pb4), bf16 moving
                t.wait_ge(s_d2, D["rq"])
                pinc(t.matmul(pb(3)[:, 0:C], ones_rb[0:1, :], rq[:, :],
                              start=True, stop=True), "bcq")
                t.wait_ge(s_d2, D["rk"])
                t.wait_ge(s_a2, A["ek2"])
                pinc(t.matmul(pb(4)[:, 0:C], ones_rb[0:1, :], rk[:, :],
                              start=True, stop=True), "bck")
                # S^T per head (bf16) into pb7
                t.wait_ge(s_d2, D["wkp2"])
                t.wait_ge(s_a2, A["srq"])
                for h in range(NH):
                    for k in range(CC):
                        mm = t.matmul(pb(7)[0:48, 48 * h:48 * (h + 1)],
                                      wkp[k][:, 48 * h:48 * (h + 1)],
                                      eqp[k][:, 48 * h:48 * (h + 1)],
                                      start=(k == 0), stop=(k == CC - 1))
                pinc(mm, "st")
                # softmax denominator (f32) into pb6
                t.wait_ge(s_a2, A["exp"])
                pinc(t.matmul(pb(6)[0:1, 0:C], ones_cf[0:48, 0:1],
                              expLT[:, :], start=True, stop=True), "den")
                # broadcast r into pb3
                t.wait_ge(s_d2, D["r"])
                pinc(t.matmul(pb(3)[:, 0:C], ones_rb[0:1, :],
                              r_row[:, :], start=True, stop=True), "bcr")
                # phase 3: blockdiag(expm1)^T via shifted-identity placements
                t.wait_ge(s_d2, D["expdev"])
                for i, (j, h, base) in enumerate(pm_list):
                    mm = t.matmul(pb(j)[:, 48 * h:48 * (h + 1)],
                                  identE[0:48, 128 - base:256 - base],
                                  expdev[0:48, 48 * h:48 * (h + 1)],
                                  start=True, stop=True)
                pinc(mm, "place")
                # M^T_dev = Wv^T AblkT (bf16)
                for m in range(CC):
                    if m == 0:
                        t.wait_ge(s_a2, A["ab2"])
                    if m == 1:
                        t.wait_ge(s_d2, D["r"])   # pb6: DVE read 1/den
                    if m == 2:
                        t.wait_ge(s_d2, D["nt0"])
                    for kv in range(CC):
                        mm = t.matmul(pb(mt_bank[m])[:, 0:C],
                                      wv[kv][:, 128 * m:128 * (m + 1)],
                                      ablkT[kv][:, :],
                                      start=(kv == 0), stop=(kv == CC - 1))
                    pinc(mm, f"mt{m}")
                if stop_after == 'ph3':
                    return

                # phase 4: out_dev = M_dev x8, fp8 DoubleRow, 2 passes/tile
                t.wait_ge(s_d2, D["nt2"])
                for grp in range(NGRP):
                    m, w = grp // NWIN, grp % NWIN
                    pt = pA if grp % 2 == 0 else pB
                    if m == 0:
                        t.wait_ge(s_x8[w], 16)
                    if grp >= 2:
                        if grp % 2 == 0:
                            t.wait_ge(s_oa, (grp - 2) // 2 + 1)
                        else:
                            t.wait_ge(s_od, (grp - 2) // 2 + 1)
                    b2, zb = blk(w) + 2, zblk(w)
                    for ns in range(4):
                        sl = pt[:, 512 * ns:512 * (ns + 1)]
                        t.matmul(sl, stA[:, :, 128 * m:128 * (m + 1)],
                                 x8b[:, blk(w):blk(w) + 2,
                                     512 * ns:512 * (ns + 1)],
                                 start=True, stop=False, perf_mode=DR)
                        t.matmul(sl, stB[:, :, 128 * m:128 * (m + 1)],
                                 x8b[:, b2:zb + 1:zb - b2,
                                     512 * ns:512 * (ns + 1)],
                                 start=False, stop=True,
                                 perf_mode=DR).then_inc(s_mm4, 1)

            # ------------- ACT: evictions + exp + sqrt ------------------
            @block.scalar
            def _(s):
                a2 = [0]

                def ainc(inst, name):
                    a2[0] += 1
                    assert A[name] == a2[0], (name, a2[0])
                    inst.then_inc(s_a2, 1)

                if stop_after == 'ph1':
                    return
                # G evictions (bf16)
                s.wait_ge(s_g, 1)
                ainc(s.copy(g_sb[0][:, :], pb(0)[:, 0:C]), "g0")
                ainc(s.copy(g_sb[1][:, 128:C], pb(1)[:, 0:C - 128]), "g1p")
                ainc(s.copy(g_sb[2][:, 256:C], pb(2)[:, 0:C - 256]), "g2p")
                s.wait_ge(s_p2, P["symt1"])
                ainc(s.copy(g_sb[1][:, 0:128], pb(3)[:, 0:128]), "sym1")
                s.wait_ge(s_p2, P["symt2"])
                ainc(s.copy(g_sb[2][:, 0:128], pb(4)[:, 0:128]), "sym2")
                s.wait_ge(s_p2, P["symt3"])
                ainc(s.copy(g_sb[2][:, 128:256], pb(5)[:, 0:128]), "sym3")
                # E evictions (f32)
                for grp in range(6):
                    s.wait_ge(s_p2, P[f"m{eg_name[grp]}"])
                    dst = eq_sb[grp] if grp < CC else ek_sb[grp - CC]
                    ainc(s.copy(dst[:, :], pb(eg_bank[grp])[:, 0:C]),
                         eg_name[grp])
                # srq = sqrt(hw*nq^2) = nq*sqrt(hw); srk = nk
                s.wait_ge(s_p2, P["nq2"])
                ainc(s.activation(srq[:, :], pb(7)[0:1, 0:C], AF.Sqrt,
                                  scale=float(HW)), "srq")
                s.wait_ge(s_p2, P["nk2"])
                ainc(s.activation(srk[:, :], pb(6)[0:1, 0:C], AF.Sqrt,
                                  scale=1.0), "srk")
                # exp of logits^T (f32)
                s.wait_ge(s_p2, P["st"])
                ainc(s.activation(expLT[:, :], pb(7)[0:48, 0:C], AF.Exp),
                     "exp")
                # den eviction (f32)
                s.wait_ge(s_p2, P["den"])
                ainc(s.copy(den_sb[:, :], pb(6)[0:1, 0:C]), "denst")
                # ablkT evictions (bf16, x SCALE), written ranges only
                s.wait_ge(s_p2, P["place"])
                s.wait_ge(s_d2, D["consts"])
                for j in range(CC):
                    lo, hi = ab_rng[j]
                    ainc(s.mul(ablkT[j][:, lo:hi], pb(j)[:, lo:hi], SCALE),
                         f"ab{j}")
                if stop_after == 'ph3':
                    return
                # phase 4: even-group evictions (full 4-bank reads)
                for grp in range(0, NGRP, 2):
                    if grp >= NOB:
                        s.wait_ge(s_stb[grp % NOB], 16 * (grp // NOB))
                    s.wait_ge(s_mm4, 4 * (grp + 1))
                    s.copy(outb[grp % NOB][:, :],
                           pA[:, 0:2048]).then_inc(s_oa, 1)

            # ------------- DVE: consts + elementwise + odd evictions ----
            @block.vector
            def _(d):
                dv = [0]

                def dinc(inst, name):
                    dv[0] += 1
                    assert D[name] == dv[0], (name, dv[0])
                    inst.then_inc(s_d2, 1)

                d.memset(x8b[:, 12, :], 0.0)
                d.memset(x8b[:, 25, :], 0.0)
                d.memset(stB[:, 1, :], 0.0)
                for j in range(CC):
                    d.memset(ablkT[j][:, :], 0.0)
                d.memset(ones_cb[:, :], 1.0)
                d.memset(ones_cf[:, :], 1.0)
                dinc(d.memset(ones_rb[:, :], 1.0), "consts")
                if stop_after == 'ph1':
                    return
                for k in range(CC):
                    d.wait_ge(s_a2, A[f"eq{k}"])
                    dinc(d.tensor_mul(zq_sb[k][:, :], eq_sb[k][:, :],
                                      wT[k][:, 0:C]), f"zq{k}")
                for k in range(CC):
                    d.wait_ge(s_a2, A[f"ek{k}"])
                    dinc(d.tensor_mul(zk_sb[k][:, :], ek_sb[k][:, :],
                                      wT[k][:, C:2 * C]), f"zk{k}")
                d.wait_ge(s_a2, A["srq"])
                with nc.allow_low_precision(reason="rq/rk are pure scales; "
                                            "0.4% scale error on ~1e-4 "
                                            "logits is negligible"):
                    dinc(d.reciprocal(rq[:, :], srq[:, :]), "rq")
                    d.wait_ge(s_a2, A["srk"])
                    dinc(d.reciprocal(rk[:, :], srk[:, :]), "rk")
                d.wait_ge(s_p2, P["bck"])
                for k in range(CC):
                    dinc(d.tensor_mul(eqp[k][:, :], eq_sb[k][:, :],
                                      pb(3)[:, 0:C]), f"eqp{k}")
                for k in range(CC):
                    dinc(d.tensor_mul(wkp[k][:, :], wT[k][:, C:2 * C],
                                      pb(4)[:, 0:C]), f"wkp{k}")
                d.wait_ge(s_a2, A["exp"])
                dinc(d.tensor_scalar_add(expdev[:, :], expLT[:, :], -1.0),
                     "expdev")
                d.wait_ge(s_p2, P["den"])
                with nc.allow_low_precision(reason="1/den scale in bf16; "
                                            "host divides by exact f32 den"):
                    dinc(d.reciprocal(r_row[:, :], pb(6)[0:1, 0:C]), "r")
                d.wait_ge(s_p2, P["bcr"])
                dinc(d.tensor_copy(rb_sb[:, :], pb(3)[:, 0:C]), "rb")
                d.wait_ge(s_d2, D["rb"])
                # ntb: pair-packed fp8 stationary tiles, col-scaled by 1/den
                st_dst = [stA[:, 0, :], stA[:, 1, :], stB[:, 0, :]]
                for m in range(CC):
                    d.wait_ge(s_p2, P[f"mt{m}"])
                    dinc(d.tensor_mul(st_dst[m], pb(mt_bank[m])[:, 0:C],
                                      rb_sb[:, :]), f"nt{m}")
                if stop_after == 'ph3':
                    return
                # phase 4: odd-group evictions
                for grp in range(1, NGRP, 2):
                    if grp >= NOB:
                        d.wait_ge(s_stb[grp % NOB], 16 * (grp // NOB))
                    d.wait_ge(s_mm4, 4 * (grp + 1))
                    d.tensor_copy(outb[grp % NOB][:, :],
                                  pB[:, 0:2048]).then_inc(s_od, 1)

            # ------------- SP: w loads + den store + out stores ---------
            @block.sync
            def _(sp):
                for k in range(CC):
                    sp.dma_start(out=wT[k][:, :],
                                 in_=wt_d[128 * k:128 * (k + 1), :]
                                 ).then_inc(s_w, 16)
                for k in range(CC):
                    sp.dma_start(out=wv[k][:, :],
                                 in_=wv_d[128 * k:128 * (k + 1), :]
                                 ).then_inc(s_w, 16)
                if stop_after != 'full':
                    return
                sp.wait_ge(s_a2, A["denst"])
                sp.dma_start(out=den_d[:, :], in_=den_sb[:, :]
                             ).then_inc(s_den, 16)
                for grp in range(NGRP):
                    m, w = grp // NWIN, grp % NWIN
                    if grp % 2 == 0:
                        sp.wait_ge(s_oa, grp // 2 + 1)
                    else:
                        sp.wait_ge(s_od, (grp - 1) // 2 + 1)
                    sp.dma_start(
                        out=od_d[128 * m:128 * (m + 1),
                                 WIN * w:WIN * (w + 1)],
                        in_=outb[grp % NOB][:, :]).then_inc(
                            s_stb[grp % NOB], 16)
                for i in range(NOB):
                    cnt = len([g for g in range(NGRP) if g % NOB == i])
                    sp.wait_ge(s_stb[i], 16 * cnt)
                sp.wait_ge(s_den, 16)

    return nc


_cache = {}


def _get_nc():
    if 'nc' not in _cache:
        _cache['nc'] = build_bass()
    return _cache['nc']


def host_pack(x, w_qkv):
    """x: [B, 384, 128, 128] f32, w_qkv: [1152, 384] f32 -> per-core input
    maps + host-side B matrix [B, 8, 16384]."""
    B = x.shape[0]
    x2 = np.ascontiguousarray(x.reshape(B, C, HW), dtype=np.float32)
    w = np.ascontiguousarray(w_qkv, dtype=np.float32)
    x8 = x2.astype(E4)                                   # [B, 384, 16384]
    # xt8[b, p, 768c + 384i + d] = x8[b, d, 256c + 128i + p]
    t = np.asarray(x8).reshape(B, C, NC64, 2, 128)
    xt8 = np.ascontiguousarray(t.transpose(0, 4, 2, 3, 1)).reshape(
        B, 128, NC64 * 2 * C)
    wt_h = np.ascontiguousarray(w.T).astype(BF)          # [384, 1152]
    wv_h = np.ascontiguousarray(w[2 * C:3 * C]).astype(BF)   # [384, 384]
    U = w[2 * C:3 * C].reshape(NH, HC, C).sum(axis=1)    # [8, 384]
    Bm = np.einsum('hc,bcn->bhn', U.astype(np.float32), x2)  # [B, 8, hw]
    in_maps = [{"xt8": xt8[b], "x8": np.asarray(x8[b]),
                "wt": wt_h, "wv": wv_h} for b in range(B)]
    return in_maps, Bm


def host_combine(Bm, dens, outdevs):
    """Combine base + deviation terms -> [B, 384, 128, 128] f32."""
    B = Bm.shape[0]
    head_of_c = np.repeat(np.arange(NH), HC)
    outs = np.empty((B, C, HW), dtype=np.float32)
    for b in range(B):
        den = np.asarray(dens[b], dtype=np.float32).reshape(C)
        od = np.asarray(outdevs[b]).astype(np.float32)
        outs[b] = Bm[b][head_of_c, :] / den[:, None] + od * (1.0 / SCALE)
    return outs.reshape(B, C, 128, 128)


def kernel(x, w_qkv):
    """x: [8, 384, 128, 128] f32, w_qkv: [1152, 384] f32 ->
    out: [8, 384, 128, 128] f32. Batch-parallel over 8 NeuronCores."""
    x = np.ascontiguousarray(x, dtype=np.float32)
    w_qkv = np.ascontiguousarray(w_qkv, dtype=np.float32)
    B = x.shape[0]
    nc = _get_nc()
    in_maps, Bm = host_pack(x, w_qkv)
    res = run_bass_kernel_spmd(nc, in_maps, list(range(B)))
    dens = [res.results[b]["den"] for b in range(B)]
    outdevs = [res.results[b]["outdev"] for b in range(B)]
    return host_combine(Bm, dens, outdevs).astype(np.float32)
